# revision 1
# baseline (speedup 1.0000x reference)
"""Trainium2 Bass kernel: 3x3 SAME conv (NCHW/OIHW) + bias.

Full problem: inp (32,128,56,56) f32, kernel (256,128,3,3) f32, bias (256,) f32
-> out (32,256,56,56) f32.

Strategy: data-parallel over batch across 8 cores (4 images/core). Host-side
prep inside kernel(): zero-pad images to [128,58,58], transpose weights to
[C,O,9], reshape bias to [128,2] — every device DMA is contiguous. Per image,
implicit GEMM: contraction K = C_in = 128 on the partition dim, M = 128 output
channels per tile (2 tiles), moving N = 448 spatial pixels (8 output rows x 56
cols via a strided AP over the padded SBUF image). The 9 kernel taps
accumulate into one PSUM bank; the PSUM->SBUF drain on the scalar engine fuses
the bias add.

DT_MODE selects the PE datapath:
  fp32  — exact (4 cycles/row)
  fp32r — single-pass fp32 mode, 1 cycle/row, ~1e-4 rel err   (default)
  bf16  — host-precast bf16, fused LDW+MM
  bf16s — bf16 with one explicit LDWEIGHTS per (otile, tap) reused by the 7
          row-chunk matmuls (tap-outer order, 7 PSUM banks live)
"""

import os as _os
from contextlib import ExitStack

import numpy as np

import concourse.bass as bass
import concourse.tile as tile
from concourse import bacc, mybir
from concourse.bass_utils import run_bass_kernel_spmd
from concourse.tile import add_dep_helper

N_CORES = 8
B_FULL, C, H, W = 32, 128, 56, 56
O = 256
KH = KW = 3
B = B_FULL // N_CORES          # images per core
PH, PW = H + 2, W + 2          # zero-padded image dims
CHUNK = 8                      # output rows per matmul group
NCHUNK = H // CHUNK            # 7
OTILES = O // 128              # 2
FDIM = CHUNK * W               # 448 moving elements per matmul

DT_MODE = _os.environ.get("K_DT", "fp16s")   # fp32|fp32r|bf16|bf16s|fp16s
REPS = int(_os.environ.get("K_REPS", "1"))   # device-side repeat (timing)
PREWARM = int(_os.environ.get("K_PREWARM", "4"))

PSUM_BUFS = int(_os.environ.get("K_PSUM_BUFS", "6"))
OUT_BUFS = int(_os.environ.get("K_OUT_BUFS", "12"))
PAD_BUFS = int(_os.environ.get("K_PAD_BUFS", "2"))
CONTIG = _os.environ.get("K_CONTIG", "0") == "1"  # contiguous rhs w/ junk cols
CHUNK = int(_os.environ.get("K_CHUNK", str(CHUNK)))
NCHUNK = H // CHUNK
FDIM = CHUNK * (PW if CONTIG else W)

_CD = {"fp32": mybir.dt.float32, "fp32r": mybir.dt.float32r,
       "bf16": mybir.dt.bfloat16, "bf16s": mybir.dt.bfloat16,
       "fp16s": mybir.dt.float16}
CAST_MODES = ("bf16", "bf16s", "fp16s")     # host-precast, explicit-LDW-capable
TAPOUT_MODES = ("bf16s", "fp16s")           # tap-outer, explicit LDWEIGHTS


def conv_body(ctx: ExitStack, tc: tile.TileContext, out: bass.AP, inp: bass.AP,
              ker: bass.AP, bias: bass.AP):
    """inp [B, C, PH, PW] pre-padded; ker [C, O, 9]; bias [128, OTILES];
    out [B, O, H, W]. inp/ker DRAM dtype: bf16 for bf16 modes else fp32."""
    nc = tc.nc
    cd = _CD[DT_MODE]
    bitcast = DT_MODE == "fp32r"   # DRAM fp32 bits reinterpreted as fp32r

    def as_cd(ap):
        return ap.bitcast(cd) if bitcast else ap

    singles = ctx.enter_context(tc.tile_pool(name="singles", bufs=1))
    if DT_MODE in TAPOUT_MODES:
        # 8 persistent PSUM bank tiles with per-bank tags: dependency
        # tracking is then per-bank (drain of THIS bank), not a pool-wide
        # watermark that gates a new group's matmuls on the previous
        # group's last drain.
        psb_pool = ctx.enter_context(
            tc.tile_pool(name="psbank", bufs=1, space="PSUM"))
        ps_banks = [psb_pool.tile([128, FDIM], mybir.dt.float32,
                                  name=f"psb{i}", tag=f"psb{i}")
                    for i in range(8)]
        ps_seq = [0]
        # PE pre-warm: a few dummy matmuls on zeroed SBUF during the input
        # DMA wait keep the HAM activity window busy so the first real
        # matmuls run at full clock (and, in the sim, full p-state).
        if PREWARM:
            warm = singles.tile([128, 576], cd, name="warm", tag="warm")
            nc.vector.memset(warm[:], 0)
            for _ in range(PREWARM):
                nc.tensor.matmul(ps_banks[7][:], warm[:, :128],
                                 warm[:, 128:128 + FDIM],
                                 start=True, stop=True)
    else:
        psum_pool = ctx.enter_context(
            tc.tile_pool(name="psum", bufs=PSUM_BUFS, space="PSUM"))
    out_pool = ctx.enter_context(tc.tile_pool(name="outs", bufs=OUT_BUFS))

    # Weights [c, o, tap] — first DMA issued (sync queue) so the serial DMA
    # fabric delivers them before image 0's rows; the first LDW needs both.
    w_sb = singles.tile([C, O, KH * KW], cd)
    nc.sync.dma_start(out=w_sb[:], in_=as_cd(ker[:]))

    # Bias [p, otile]: bias for output channel ot*128+p.
    b_sb = singles.tile([128, OTILES], mybir.dt.float32)
    nc.sync.dma_start(out=b_sb[:], in_=bias)

    # Padded image buffers, rotated across images; fully written by each DMA.
    # CONTIG streams run up to 2 elements past the image end — allocate spare
    # and initialize it once (values are junk-lane only, never read as output).
    flat_len = PH * PW + (2 if CONTIG else 0)
    pads = [singles.tile([C, flat_len] if CONTIG else [C, PH, PW], cd,
                         name=f"pad{i}", tag=f"pad{i}")
            for i in range(PAD_BUFS)]
    if CONTIG:
        for p in pads:
            nc.scalar.dma_start(out=p[:, PH * PW:], in_=as_cd(inp[0])
                                .rearrange("c h w -> c (h w)")[:, :2])

    def drain(n, ot, chunk, ps, eng="s", split=False):
        y0 = chunk * CHUNK
        o_sb = out_pool.tile([128, FDIM], mybir.dt.float32, name="o_sb",
                             tag="o_sb")
        if split:
            # Kernel tail: halve latency by pairing ACT + DVE on one chunk.
            h = FDIM // 2
            nc.scalar.activation(o_sb[:, :h], ps[:, :h],
                                 mybir.ActivationFunctionType.Identity,
                                 bias=b_sb[:, ot:ot + 1])
            nc.vector.tensor_scalar_add(o_sb[:, h:], ps[:, h:],
                                        b_sb[:, ot:ot + 1])
        elif eng == "v":
            nc.vector.tensor_scalar_add(o_sb[:], ps[:], b_sb[:, ot:ot + 1])
        else:
            nc.scalar.activation(o_sb[:], ps[:],
                                 mybir.ActivationFunctionType.Identity,
                                 bias=b_sb[:, ot:ot + 1])
        o_eng = nc.sync if (chunk % 2 == 0) else nc.scalar
        o_src = o_sb[:]
        if CONTIG:
            o_src = o_src.rearrange("c (r w) -> c r w", w=PW)[:, :, :W]
        o_eng.dma_start(out=out[n, ot * 128:(ot + 1) * 128, y0:y0 + CHUNK, :],
                        in_=o_src)

    def rhs_ap(p_in, chunk, tap):
        dy, dx = tap // KW, tap % KW
        y0 = chunk * CHUNK
        if CONTIG:
            start = (y0 + dy) * PW + dx
            return p_in[:, start:start + FDIM]
        return p_in[:, y0 + dy:y0 + dy + CHUNK, dx:dx + W]

    def one_image(n):
        p_in = pads[n % PAD_BUFS]
        i_src = as_cd(inp[n])
        half = PH // 2
        if CONTIG:
            i_flat = i_src.rearrange("c h w -> c (h w)")
            nc.sync.dma_start(out=p_in[:, :half * PW],
                              in_=i_flat[:, :half * PW])
            nc.gpsimd.dma_start(out=p_in[:, half * PW:PH * PW],
                                in_=i_flat[:, half * PW:])
        elif n == 0 and DT_MODE in TAPOUT_MODES:
            # Image 0 is the critical path: subgroup A only needs padded
            # rows 0..33, so land those first (scalar ring, while sync
            # carries the weights) and the rest behind them.
            cut = 4 * CHUNK + 2
            nc.scalar.dma_start(out=p_in[:, :cut, :], in_=i_src[:, :cut, :])
            nc.gpsimd.dma_start(out=p_in[:, cut:, :], in_=i_src[:, cut:, :])
        else:
            nc.sync.dma_start(out=p_in[:, :half, :], in_=i_src[:, :half, :])
            nc.gpsimd.dma_start(out=p_in[:, half:, :], in_=i_src[:, half:, :])
        for ot in range(OTILES):
            w_ot = w_sb[:, ot * 128:(ot + 1) * 128, :]
            if DT_MODE in TAPOUT_MODES:
                # Two subgroups per (image, otile): subgroup A's drains run
                # while subgroup B's matmuls stream, so PSUM banks are free
                # again long before the next group's tap-0 round needs them.
                for sub in (range(0, 4), range(4, NCHUNK)):
                    sub = list(sub)
                    pss = {}
                    for chunk in sub:
                        pss[chunk] = ps_banks[ps_seq[0] % 8]
                        ps_seq[0] += 1
                    prev_pe = None
                    for tap in range(KH * KW):
                        ldw = nc.tensor.ldweights(w_ot[:, :, tap])
                        if prev_pe is not None:
                            add_dep_helper(ldw.ins, prev_pe.ins, False,
                                           "ldw after prev tap's matmuls")
                        for chunk in sub:
                            mm = nc.tensor.matmul(
                                pss[chunk][:], w_ot[:, :, tap],
                                rhs_ap(p_in, chunk, tap),
                                start=(tap == 0), stop=(tap == KH * KW - 1))
                            mm.ins.ldweights = False
                            add_dep_helper(mm.ins, ldw.ins, False,
                                           "matmul uses explicit ldweights")
                            prev_pe = mm
                    last = (n == B - 1 and ot == OTILES - 1
                            and sub[-1] == NCHUNK - 1)
                    for chunk in sub:
                        drain(n, ot, chunk, pss[chunk],
                              eng="v" if chunk % 2 else "s", split=last)
            else:
                for chunk in range(NCHUNK):
                    ps = psum_pool.tile([128, FDIM], mybir.dt.float32,
                                        name="ps", tag="ps")
                    for tap in range(KH * KW):
                        nc.tensor.matmul(ps[:], w_ot[:, :, tap],
                                         rhs_ap(p_in, chunk, tap),
                                         start=(tap == 0),
                                         stop=(tap == KH * KW - 1))
                    drain(n, ot, chunk, ps)

    def body():
        for n in range(B):
            one_image(n)

    reps = getattr(tc, "_k_reps", REPS)
    if reps > 1:
        with tc.For_i(0, reps, 1):
            body()
    else:
        body()


WCH = 14                      # wino: output rows per chunk group
WG = H // WCH                 # 4
NT = W // 2                   # 28 column tiles (2 outputs each)
WF = WCH * NT                 # 392 moving elements per wino matmul


def wino_body(ctx: ExitStack, tc: tile.TileContext, out: bass.AP,
              inp: bass.AP, ker: bass.AP, bias: bass.AP):
    """1-D Winograd F(2,3) along W, fp16. inp [B,C,58,58] fp16 pre-padded;
    ker [C,O,12] fp16 = G-transformed taps (p,dy); bias [128,2] f32.

    Per column-tile tx (28 of them): V planes v0=d0-d2, v1=d1+d2, v2=d2-d1,
    v3=d1-d3 over even/odd padded cols. m_p = sum_dy W~[p,dy]^T V_p[y+dy].
    out_even = m0+m1+m2+bias, out_odd = m1-m2-m3+bias. 12 matmuls of 392
    rows replace 18 of 448 per (img,ot,14-row group): PE rows x2/3.
    """
    nc = tc.nc
    cd = mybir.dt.float16
    f32 = mybir.dt.float32

    singles = ctx.enter_context(tc.tile_pool(name="singles", bufs=1))
    psb_pool = ctx.enter_context(
        tc.tile_pool(name="psbank", bufs=1, space="PSUM"))
    ps_banks = [psb_pool.tile([128, WF], f32, name=f"psb{i}", tag=f"psb{i}")
                for i in range(8)]
    ps_seq = [0]
    vpool = ctx.enter_context(tc.tile_pool(name="vplanes", bufs=2))
    tpool = ctx.enter_context(tc.tile_pool(name="ttmp", bufs=8))
    out_pool = ctx.enter_context(tc.tile_pool(name="outs", bufs=OUT_BUFS))

    if PREWARM:
        warm = singles.tile([128, 128 + WF], cd, name="warm", tag="warm")
        nc.vector.memset(warm[:], 0)
        for _ in range(PREWARM):
            nc.tensor.matmul(ps_banks[7][:], warm[:, :128], warm[:, 128:],
                             start=True, stop=True)

    w_sb = singles.tile([C, O, 12], cd)
    nc.sync.dma_start(out=w_sb[:], in_=ker[:])
    b_sb = singles.tile([128, OTILES], f32)
    nc.sync.dma_start(out=b_sb[:], in_=bias)
    pads = [singles.tile([C, PH, PW], cd, name=f"pad{i}", tag=f"pad{i}")
            for i in range(PAD_BUFS)]

    add, sub = mybir.AluOpType.add, mybir.AluOpType.subtract

    def one_image(n):
        p_in = pads[n % PAD_BUFS]
        i_src = inp[n]
        if n == 0:
            cut = 2 * WCH + 2
            nc.scalar.dma_start(out=p_in[:, :cut, :], in_=i_src[:, :cut, :])
            nc.gpsimd.dma_start(out=p_in[:, cut:, :], in_=i_src[:, cut:, :])
        else:
            half = PH // 2
            nc.sync.dma_start(out=p_in[:, :half, :], in_=i_src[:, :half, :])
            nc.gpsimd.dma_start(out=p_in[:, half:, :],
                                in_=i_src[:, half:, :])
        d0 = p_in[:, :, 0:55:2]
        d1 = p_in[:, :, 1:56:2]
        d2 = p_in[:, :, 2:57:2]
        d3 = p_in[:, :, 3:58:2]
        v = vpool.tile([C, 4, PH, NT], cd, name="v", tag="v")
        nc.vector.tensor_tensor(out=v[:, 0], in0=d0, in1=d2, op=sub)
        nc.gpsimd.tensor_tensor(out=v[:, 1], in0=d1, in1=d2, op=add)
        nc.gpsimd.tensor_tensor(out=v[:, 2], in0=d2, in1=d1, op=sub)
        nc.gpsimd.tensor_tensor(out=v[:, 3], in0=d1, in1=d3, op=sub)
        for ot in range(OTILES):
            w_ot = w_sb[:, ot * 128:(ot + 1) * 128, :]
            for g in range(WG):
                y0 = WCH * g
                ps = []
                for _ in range(4):
                    ps.append(ps_banks[ps_seq[0] % 8])
                    ps_seq[0] += 1
                for p in range(4):
                    for dy in range(3):
                        nc.tensor.matmul(
                            ps[p][:], w_ot[:, :, p * 3 + dy],
                            v[:, p, y0 + dy:y0 + dy + WCH, :],
                            start=(dy == 0), stop=(dy == 2))
                o_sb = out_pool.tile([128, WCH, W], f32, name="o_sb",
                                     tag="o_sb")
                t0 = tpool.tile([128, WF], f32, name="t0", tag="t0")
                t1 = tpool.tile([128, WF], f32, name="t1", tag="t1")
                t2 = tpool.tile([128, WF], f32, name="t2", tag="t2")
                t3 = tpool.tile([128, WF], f32, name="t3", tag="t3")
                nc.vector.tensor_tensor(out=t0[:], in0=ps[0][:], in1=ps[1][:],
                                        op=add)
                nc.vector.tensor_tensor(out=t1[:], in0=t0[:], in1=ps[2][:],
                                        op=add)
                nc.vector.tensor_tensor(out=t2[:], in0=ps[1][:], in1=ps[2][:],
                                        op=sub)
                nc.vector.tensor_tensor(out=t3[:], in0=t2[:], in1=ps[3][:],
                                        op=sub)
                t1v = t1[:].rearrange("c (r t) -> c r t", t=NT)
                t3v = t3[:].rearrange("c (r t) -> c r t", t=NT)
                nc.scalar.activation(o_sb[:, :, 0:55:2], t1v,
                                     mybir.ActivationFunctionType.Identity,
                                     bias=b_sb[:, ot:ot + 1])
                nc.scalar.activation(o_sb[:, :, 1:56:2], t3v,
                                     mybir.ActivationFunctionType.Identity,
                                     bias=b_sb[:, ot:ot + 1])
                o_eng = nc.sync if g % 2 == 0 else nc.scalar
                o_eng.dma_start(
                    out=out[n, ot * 128:(ot + 1) * 128, y0:y0 + WCH, :],
                    in_=o_sb[:])

    def body():
        for n in range(B):
            one_image(n)

    reps = getattr(tc, "_k_reps", REPS)
    if reps > 1:
        with tc.For_i(0, reps, 1):
            body()
    else:
        body()


def build_nc(reps: int | None = None) -> bass.Bass:
    wino = DT_MODE == "wino"
    if wino:
        in_dt, ntaps = mybir.dt.float16, 12
    else:
        in_dt = _CD[DT_MODE] if DT_MODE in CAST_MODES else mybir.dt.float32
        ntaps = KH * KW
    nc = bacc.Bacc(trn_type="TRN2", target_bir_lowering=False, debug=False)
    inp = nc.dram_tensor("inp", [B, C, PH, PW], in_dt,
                         kind="ExternalInput").ap()
    ker = nc.dram_tensor("kernel", [C, O, ntaps], in_dt,
                         kind="ExternalInput").ap()
    bias = nc.dram_tensor("bias", [128, OTILES], mybir.dt.float32,
                          kind="ExternalInput").ap()
    out = nc.dram_tensor("out", [B, O, H, W], mybir.dt.float32,
                         kind="ExternalOutput").ap()
    with tile.TileContext(nc) as tc:
        if reps is not None:
            tc._k_reps = reps
        with ExitStack() as ctx:
            (wino_body if wino else conv_body)(ctx, tc, out, inp, ker, bias)
    nc.compile()
    return nc


def host_prep(inp, kernel, bias):
    """Shard-side layout prep: pad + transpose + cast to the DRAM dtypes."""
    inp = np.ascontiguousarray(inp, dtype=np.float32)
    kernel = np.ascontiguousarray(kernel, dtype=np.float32)
    bias = np.ascontiguousarray(bias, dtype=np.float32)
    if DT_MODE in ("fp16s", "wino"):
        np_dt = np.float16
    elif DT_MODE in ("bf16", "bf16s"):
        import ml_dtypes
        np_dt = ml_dtypes.bfloat16
    else:
        np_dt = np.float32
    inp_pad = np.zeros((B_FULL, C, PH, PW), np_dt)
    inp_pad[:, :, 1:1 + H, 1:1 + W] = inp
    if DT_MODE == "wino":
        k_t = kernel.transpose(1, 0, 2, 3)          # [C, O, dy, dx]
        wt = np.empty((C, O, 4, KH), np.float32)    # [c, o, p, dy]
        g0, g1, g2 = k_t[..., 0], k_t[..., 1], k_t[..., 2]
        wt[:, :, 0, :] = g0
        wt[:, :, 1, :] = (g0 + g1 + g2) * 0.5
        wt[:, :, 2, :] = (g0 - g1 + g2) * 0.5
        wt[:, :, 3, :] = g2
        w_host = np.ascontiguousarray(wt.reshape(C, O, 12)).astype(np_dt)
    else:
        w_host = np.ascontiguousarray(
            kernel.transpose(1, 0, 2, 3).reshape(C, O, KH * KW)).astype(np_dt)
    b_host = np.ascontiguousarray(bias.reshape(OTILES, 128).T)
    return inp_pad, w_host, b_host


_NC_CACHE = None


def kernel(inp: np.ndarray, kernel: np.ndarray, bias: np.ndarray) -> np.ndarray:
    global _NC_CACHE
    if _NC_CACHE is None:
        _NC_CACHE = build_nc()
    nc = _NC_CACHE
    inp_pad, w_host, b_host = host_prep(inp, kernel, bias)
    in_maps = [
        {"inp": inp_pad[i * B:(i + 1) * B], "kernel": w_host, "bias": b_host}
        for i in range(N_CORES)
    ]
    res = run_bass_kernel_spmd(nc, in_maps, core_ids=list(range(N_CORES)))
    return np.concatenate([r["out"] for r in res.results], axis=0)

